# revision 33
# baseline (speedup 1.0000x reference)
"""ChebConv (K=3) kernel for Trainium2, data-parallel over batch across 8 NeuronCores.

Math (per batch b), monomial expansion of the Chebyshev recurrence:
    d    = adj.sum(axis=1),  dinv = (d+eps)^-1/2,  dinv2 = dinv^2
    M    = Dinv A Dinv   (L = I - M)
    out  = relu(x V0 + M x V1 + M^2 x V2 + bsum)
  with   V0 = W0+W1+W2,  V1 = -(W1+4 W2),  V2 = 2 W2   (host-precomputed)
  and    M x  = dinv . w1,  w1 = A y0,  y0 = dinv . x
         M^2x = dinv . w2,  w2 = A y1,  y1 = dinv2 . w1

Device schedule:
  - A streams in as 16 row-strips (fp32); a fused DVE/ACT op casts to bf16
    and row-sum-reduces (accum_out) in one pass.
  - Each strip is transposed on the PE against a constant identity (no
    dependency on the dinv chain), filling ats[j, i] = A[i, j] (bf16).
  - Pass 1 (z1T[f, i] = sum_c y0[c] x ats[c, i-blk]) is triangular over
    (c, i-block) pairs: term (c, g) issues once strip c (for y0[c]) and
    strip group g (for the ats columns) are both in.  Every column block
    still completes only after the last strip (y0[15] gates them all).
  - Tail: pass-1 rim, then per group: z1 evac (bf16 z1bf == w1^T, reused
    as output stationary), 4 PE transposes -> w1-natural, dinv2-scale ->
    y1, then that group's 16 pass-2 terms; z2 groups evac as they finish
    and feed the output matmuls, all pipelined.
  - Output: out[i-chunk] = relu(psa + dinv.(w1T V1 + w2T V2)) where
    psa = xT V0 + bsum (rank-1 matmul trick) is precomputed mid-stream.
"""

import numpy as np

B, N, F, K = 8, 2048, 128, 3
P = 128
NT = N // P    # 16 strips
NG = 4         # column groups of 4 strips (512 cols)
EPS = 1e-6
NCORES = 8

_cache = {}


def _build_nc():
    from contextlib import ExitStack

    import concourse.bacc as bacc
    import concourse.tile as tile
    from concourse import mybir

    f32 = mybir.dt.float32
    bf16 = mybir.dt.bfloat16
    AF = mybir.ActivationFunctionType
    OP = mybir.AluOpType

    nc = bacc.Bacc("TRN2", target_bir_lowering=False, debug=False, num_devices=NCORES)
    adj = nc.dram_tensor("adj", [N, N], f32, kind="ExternalInput").ap()
    x = nc.dram_tensor("x", [P, NT * F], f32, kind="ExternalInput").ap()
    v_d = nc.dram_tensor("v", [K, F, F], f32, kind="ExternalInput").ap()
    brow_d = nc.dram_tensor("brow", [P, F], f32, kind="ExternalInput").ap()
    e0row_d = nc.dram_tensor("e0row", [P, P], f32, kind="ExternalInput").ap()
    ident_d = nc.dram_tensor("ident", [P, P], f32, kind="ExternalInput").ap()
    out = nc.dram_tensor("out", [N, F], f32, kind="ExternalOutput").ap()
    out_t = out.rearrange("(t p) f -> p t f", p=P)

    with ExitStack() as ctx:
        tc = ctx.enter_context(tile.TileContext(nc))
        consts = ctx.enter_context(tc.tile_pool(name="consts", bufs=1))
        afp = ctx.enter_context(tc.tile_pool(name="afp", bufs=8))
        abp = ctx.enter_context(tc.tile_pool(name="abp", bufs=5))
        big = ctx.enter_context(tc.tile_pool(name="big", bufs=1))
        small = ctx.enter_context(tc.tile_pool(name="small", bufs=4))
        ps_t = ctx.enter_context(tc.tile_pool(name="ps_t", bufs=2, space="PSUM"))
        ps_z = ctx.enter_context(tc.tile_pool(name="ps_z", bufs=4, space="PSUM"))

        # ---- input DMAs: x + consts on the ACT HWDGE ring so they run
        # alongside the adjacency strips on the SP ring (gpsimd/SWDGE is
        # avoided entirely -- its first use costs a ~15us drain) ---------
        ident_f = consts.tile([P, P], f32)
        nc.scalar.dma_start(out=ident_f, in_=ident_d)
        v_f = consts.tile([P, K, F], f32)
        nc.scalar.dma_start(out=v_f, in_=v_d.rearrange("k i o -> i k o"))
        e0_f = consts.tile([P, P], f32)
        nc.scalar.dma_start(out=e0_f, in_=e0row_d)
        brow_f = consts.tile([P, F], f32)
        nc.scalar.dma_start(out=brow_f, in_=brow_d)
        x_nat = big.tile([P, NT, F], f32)
        nc.scalar.dma_start(out=x_nat, in_=x)
        ident_bf = consts.tile([P, P], bf16)
        nc.scalar.copy(out=ident_bf, in_=ident_f)
        v_bf = consts.tile([P, K, F], bf16)
        nc.scalar.copy(out=v_bf, in_=v_f)
        e0_bf = consts.tile([P, P], bf16)
        nc.scalar.copy(out=e0_bf, in_=e0_f)
        brow_bf = consts.tile([P, F], bf16)
        nc.scalar.copy(out=brow_bf, in_=brow_f)
        eps_sb = consts.tile([P, 1], f32)
        nc.vector.memset(eps_sb, EPS)

        a_fs = {}

        def issue_strip_dma(r):
            a_f = afp.tile([P, N], f32, tag="af")
            nc.sync.dma_start(out=a_f, in_=adj[r * P:(r + 1) * P, :])
            a_fs[r] = a_f

        for r in range(8):
            issue_strip_dma(r)

        d_all = consts.tile([P, NT], f32)
        dsq_all = consts.tile([P, NT], f32)
        dinv = consts.tile([P, NT], f32)
        dinv2 = consts.tile([P, NT], f32)

        y0 = big.tile([P, NT, F], bf16)
        y1 = big.tile([P, NT, F], bf16)
        x_bf = big.tile([P, NT, F], bf16)
        xT = big.tile([P, NT, P], bf16)
        z1bf = big.tile([P, N], bf16)    # w1^T  [f, i]
        z2bf = big.tile([P, N], bf16)    # w2^T  [f, i]
        psa_sb = big.tile([P, NT, F], f32)  # x V0 + bsum, per i-chunk
        ats = big.tile([P, NT, N], bf16)    # [j, c, i] = A[i, j in block c]

        # ---- x prep: cast, then 16 PE transposes (warms the PE) --------
        nc.scalar.copy(out=x_bf, in_=x_nat)
        for g8 in range(2):
            pt = ps_t.tile([P, 8, P], f32, tag="t")
            for q in range(8):
                r = 8 * g8 + q
                nc.tensor.matmul(pt[:, q, :], lhsT=x_bf[:, r, :], rhs=ident_bf,
                                 start=True, stop=True)
            if g8 == 0:
                nc.vector.tensor_copy(out=xT[:, 0:8, :], in_=pt)
            else:
                nc.scalar.copy(out=xT[:, 8:16, :], in_=pt)

        # psa = xT V0 + bsum for all chunks (independent of A; runs in
        # whatever PE/ACT slack the stream leaves)
        def psa_pair(ch):
            pa = ps_t.tile([P, 2, F], f32, tag="t")
            for k in range(2):
                nc.tensor.matmul(pa[:, k, :], lhsT=xT[:, ch + k, :],
                                 rhs=v_bf[:, 0, :], start=True, stop=False,
                                 skip_group_check=True)
                nc.tensor.matmul(pa[:, k, :], lhsT=e0_bf, rhs=brow_bf,
                                 start=False, stop=True, skip_group_check=True)
            nc.scalar.copy(out=psa_sb[:, ch:ch + 2, :], in_=pa)

        # pass-1 bookkeeping: term (c, g) -> z1 column block g
        z1p = [ps_z.tile([P, 512], f32, tag=f"zg{g}", bufs=1,
                           name=f"z1p{g}") for g in range(NG)]
        p1_count = [0] * NG
        p1_issued = set()

        N_P1 = {g: NT for g in range(NG)}
        N_P1[3] = 2 * NT - 1  # 15 x 384-wide + 1 x 512 (c=15) + 15 x 128-wide

        def _p1_mm(c, g, col0, width):
            nc.tensor.matmul(z1p[g][:, col0:col0 + width],
                             lhsT=y0[:, c, :],
                             rhs=ats[:, c, g * 512 + col0:g * 512 + col0 + width],
                             start=(p1_count[g] == 0),
                             stop=(p1_count[g] == N_P1[g] - 1),
                             skip_group_check=True)
            p1_count[g] += 1

        def p1_term(c, g):
            if (c, g) in p1_issued:
                return
            p1_issued.add((c, g))
            _p1_mm(c, g, 0, 512)

        def p1_term_a(c):
            # group-3 columns from strips 12-14 only (available pre-tail)
            if (c, 3) in p1_issued:
                return
            p1_issued.add((c, 3))
            _p1_mm(c, 3, 0, 384)

        def p1_term_b(c):
            # group-3 remainder (strip 15 columns)
            if (c, "3b") in p1_issued:
                return
            p1_issued.add((c, "3b"))
            _p1_mm(c, 3, 384, 128)

        # ---- streaming phase ------------------------------------------
        # Engine FIFOs are strict; keep each engine\'s queue dependency-
        # monotone: DVE runs the casts (arrival order) plus tiny follow-ups,
        # ACT runs all PSUM evacs (PE completion order) plus dsq/y0/psa.
        for r in range(NT):
            a_f = a_fs[r]
            a_t = abp.tile([P, N], bf16, tag="a")
            d_r = small.tile([P, 1], f32, tag="d", bufs=8)
            nc.vector.tensor_scalar(
                out=a_t, in0=a_f, scalar1=1.0, scalar2=0.0, op0=OP.mult,
                op1=OP.add, accum_out=d_r)
            dsq_r = small.tile([P, 1], f32, tag="dsq", bufs=8)
            nc.scalar.activation(out=dsq_r, in_=d_r, func=AF.Sqrt, bias=eps_sb)
            nc.vector.reciprocal(out=dinv[:, r:r + 1], in_=dsq_r)
            nc.vector.tensor_mul(out=dinv2[:, r:r + 1], in0=dinv[:, r:r + 1],
                                 in1=dinv[:, r:r + 1])
            nc.scalar.activation(out=y0[:, r, :], in_=x_nat[:, r, :],
                                 func=AF.Identity, scale=dinv[:, r:r + 1])
            if r + 8 < NT:
                issue_strip_dma(r + 8)

            # transpose the strip: 16 (128x128) PE transposes vs identity;
            # two PSUM banks per evac op, all evacs on ACT
            for g8 in range(2):
                pt = ps_t.tile([P, 8, P], f32, tag="t")
                for q in range(8):
                    c = 8 * g8 + q
                    nc.tensor.matmul(pt[:, q, :], lhsT=a_t[:, c * P:(c + 1) * P],
                                     rhs=ident_bf, start=True, stop=True)
                nc.scalar.copy(
                    out=ats[:, 8 * g8:8 * g8 + 8, r * P:(r + 1) * P], in_=pt)

            if r < NT - 1:
                for gp in range((r + 1) // 4):
                    p1_term(r, gp)
                if r % 4 == 3:
                    g_new = r // 4
                    for c in range(r + 1):
                        p1_term(c, g_new)
                if r == NT - 2:
                    # strips 12-14 transposed: 384-wide group-3 terms
                    for c in range(NT - 1):
                        p1_term_a(c)
                if r >= 7:
                    psa_pair(2 * (r - 7))

        # ---- tail ------------------------------------------------------
        # Hand-interleaved so the PE never stalls on the DVE/ACT chains:
        # finish z1 group g with its (15, g) term, kick its evac, and keep
        # the PE busy on still-independent pass-1 rim terms while the
        # evac/transpose/scale chain for g runs; pass-2 terms follow as
        # their y1 blocks and z2 banks become ready.
        z2p = [None] * NG
        p2_count = [0] * NG
        p2_issued = set()

        def p2_term(c, gp):
            if (c, gp) in p2_issued or z2p[gp] is None:
                return
            p2_issued.add((c, gp))
            nc.tensor.matmul(z2p[gp], lhsT=y1[:, c, :],
                             rhs=ats[:, c, gp * 512:(gp + 1) * 512],
                             start=(p2_count[gp] == 0),
                             stop=(p2_count[gp] == NT - 1),
                             skip_group_check=True)
            p2_count[gp] += 1

        def z1_chain_pre(g):
            # evac on DVE/ACT (PE-independent)
            if g % 2 == 0:
                nc.vector.tensor_copy(out=z1bf[:, g * 512:(g + 1) * 512],
                                      in_=z1p[g])
            else:
                nc.scalar.copy(out=z1bf[:, g * 512:(g + 1) * 512], in_=z1p[g])

        def z1_chain_post(g):
            # 4 PE transposes -> natural, scale -> y1, free bank -> z2
            zn = ps_t.tile([P, 4, P], f32, tag="t", name=f"zn{g}")
            for q in range(4):
                c = 4 * g + q
                nc.tensor.matmul(zn[:, q, :], lhsT=z1bf[:, c * P:(c + 1) * P],
                                 rhs=ident_bf, start=True, stop=True)
            for q in range(4):
                c = 4 * g + q
                nc.vector.tensor_scalar(out=y1[:, c, :], in0=zn[:, q, :],
                                        scalar1=dinv2[:, c:c + 1], scalar2=None,
                                        op0=OP.mult)
            z2p[g] = ps_z.tile([P, 512], f32, tag=f"zg{g}", bufs=1,
                               name=f"z2p{g}")

        # all pass-1 terms not issued in-stream, ordered so z1 group 0
        # completes first; chains interleave to keep the PE fed
        pend = {gp: [c for c in range(NT) if (c, gp) not in p1_issued]
                for gp in range(3)}
        pend[3] = [c for c in range(NT) if (c, 3) not in p1_issued]

        for c in pend[0]:
            p1_term(c, 0)
        z1_chain_pre(0)
        for c in pend[1]:
            p1_term(c, 1)
        z1_chain_post(0)
        for c in pend[2]:
            p1_term(c, 2)
        z1_chain_pre(1)
        for c in range(4):
            p2_term(c, 0)
        rim_b = [c for c in range(NT - 1)]
        for c in pend[3]:
            p1_term(c, 3)
        for c in rim_b[:8]:
            p1_term_b(c)
        z1_chain_post(1)
        for c in rim_b[8:]:
            p1_term_b(c)
        z1_chain_pre(2)
        for gp in range(2):
            for c in range(8):
                p2_term(c, gp)
        z1_chain_post(2)
        z1_chain_pre(3)
        for gp in range(3):
            for c in range(12):
                p2_term(c, gp)
        z1_chain_post(3)
        # finish pass-2 one column group at a time; each group's output
        # (evac, psb matmuls, STT, relu, store) is issued while the PE
        # sweeps the next group's remaining terms, hiding the evac latency
        def z2_evac(g):
            if g % 2 == 0:
                nc.vector.tensor_copy(out=z2bf[:, g * 512:(g + 1) * 512],
                                      in_=z2p[g])
            else:
                nc.scalar.copy(out=z2bf[:, g * 512:(g + 1) * 512], in_=z2p[g])

        def out_group(g):
            og = small.tile([P, 4, F], f32, tag="og", bufs=2)
            tmp4 = small.tile([P, 4, F], f32, tag="tmp", bufs=2)
            for q in range(4):
                ch = 4 * g + q
                psb = ps_t.tile([P, F], f32, tag="t")
                nc.tensor.matmul(psb, lhsT=z1bf[:, ch * P:(ch + 1) * P],
                                 rhs=v_bf[:, 1, :], start=True, stop=False,
                                 skip_group_check=True)
                nc.tensor.matmul(psb, lhsT=z2bf[:, ch * P:(ch + 1) * P],
                                 rhs=v_bf[:, 2, :], start=False, stop=True,
                                 skip_group_check=True)
                nc.vector.scalar_tensor_tensor(
                    out=tmp4[:, q, :], in0=psb, scalar=dinv[:, ch:ch + 1],
                    in1=psa_sb[:, ch, :], op0=OP.mult, op1=OP.add)
            nc.scalar.activation(out=og, in_=tmp4, func=AF.Relu)
            nc.scalar.dma_start(out=out_t[:, 4 * g:4 * g + 4, :], in_=og)

        for c in range(NT):
            p2_term(c, 0)
        z2_evac(0)
        for c in range(NT):
            p2_term(c, 1)
        out_group(0)
        z2_evac(1)
        for c in range(NT):
            p2_term(c, 2)
        out_group(1)
        z2_evac(2)
        for c in range(NT):
            p2_term(c, 3)
        out_group(2)
        z2_evac(3)
        out_group(3)

    nc.compile()
    return nc

def _get_nc():
    if "nc" not in _cache:
        _cache["nc"] = _build_nc()
    return _cache["nc"]


def make_in_maps(x, adj, W, b):
    x = np.asarray(x, dtype=np.float32)
    # [B, N, F] -> [B, P, NT*F]: partition p holds x[t*128+p, :] for t in 0..15
    x_t = np.ascontiguousarray(
        x.reshape(B, NT, P, F).transpose(0, 2, 1, 3).reshape(B, P, NT * F))
    adj = np.ascontiguousarray(np.asarray(adj, dtype=np.float32))
    Wf = np.asarray(W, dtype=np.float32)
    bf = np.asarray(b, dtype=np.float32)
    V = np.ascontiguousarray(np.stack(
        [Wf[0] + Wf[1] + Wf[2], -(Wf[1] + 4.0 * Wf[2]), 2.0 * Wf[2]]))
    brow = np.zeros((P, F), dtype=np.float32)
    brow[0] = bf.sum(axis=0)
    e0row = np.zeros((P, P), dtype=np.float32)
    e0row[0] = 1.0
    ident = np.eye(P, dtype=np.float32)
    return [
        {"adj": adj[c], "x": x_t[c], "v": V, "brow": brow, "e0row": e0row,
         "ident": ident}
        for c in range(NCORES)
    ]


def run_raw(x, adj, W, b, **kwargs):
    from concourse import bass_utils

    nc = _get_nc()
    in_maps = make_in_maps(x, adj, W, b)
    res = bass_utils.run_bass_kernel_spmd(nc, in_maps,
                                          core_ids=list(range(NCORES)), **kwargs)
    out = np.stack([res.results[c]["out"] for c in range(NCORES)], axis=0)
    return out.astype(np.float32), res


def kernel(x, adj, W, b):
    out, _ = run_raw(x, adj, W, b)
    return out


# revision 34
# speedup vs baseline: 1.1235x; 1.1235x over previous
"""ChebConv (K=3) kernel for Trainium2, data-parallel over batch across 8 NeuronCores.

Math (per batch b), monomial expansion of the Chebyshev recurrence:
    d    = adj.sum(axis=1),  dinv = (d+eps)^-1/2,  dinv2 = dinv^2
    M    = Dinv A Dinv   (L = I - M)
    out  = relu(x V0 + M x V1 + M^2 x V2 + bsum)
  with   V0 = W0+W1+W2,  V1 = -(W1+4 W2),  V2 = 2 W2   (host-precomputed)
  and    M x  = dinv . w1,  w1 = A y0,  y0 = dinv . x
         M^2x = dinv . w2,  w2 = A y1,  y1 = dinv2 . w1

Device schedule:
  - A streams in as 16 row-strips (fp32); a fused DVE/ACT op casts to bf16
    and row-sum-reduces (accum_out) in one pass.
  - Each strip is transposed on the PE against a constant identity (no
    dependency on the dinv chain), filling ats[j, i] = A[i, j] (bf16).
  - Pass 1 (z1T[f, i] = sum_c y0[c] x ats[c, i-blk]) is triangular over
    (c, i-block) pairs: term (c, g) issues once strip c (for y0[c]) and
    strip group g (for the ats columns) are both in.  Every column block
    still completes only after the last strip (y0[15] gates them all).
  - Tail: pass-1 rim, then per group: z1 evac (bf16 z1bf == w1^T, reused
    as output stationary), 4 PE transposes -> w1-natural, dinv2-scale ->
    y1, then that group's 16 pass-2 terms; z2 groups evac as they finish
    and feed the output matmuls, all pipelined.
  - Output: out[i-chunk] = relu(psa + dinv.(w1T V1 + w2T V2)) where
    psa = xT V0 + bsum (rank-1 matmul trick) is precomputed mid-stream.
"""

import numpy as np

B, N, F, K = 8, 2048, 128, 3
P = 128
NT = N // P    # 16 strips
NG = 4         # column groups of 4 strips (512 cols)
EPS = 1e-6
NCORES = 8

_cache = {}


def _build_nc():
    from contextlib import ExitStack

    import concourse.bacc as bacc
    import concourse.tile as tile
    from concourse import mybir

    f32 = mybir.dt.float32
    bf16 = mybir.dt.bfloat16
    AF = mybir.ActivationFunctionType
    OP = mybir.AluOpType

    nc = bacc.Bacc("TRN2", target_bir_lowering=False, debug=False, num_devices=NCORES)
    adj = nc.dram_tensor("adj", [N, N], f32, kind="ExternalInput").ap()
    x = nc.dram_tensor("x", [P, NT * F], f32, kind="ExternalInput").ap()
    v_d = nc.dram_tensor("v", [K, F, F], f32, kind="ExternalInput").ap()
    brow_d = nc.dram_tensor("brow", [P, F], f32, kind="ExternalInput").ap()
    e0row_d = nc.dram_tensor("e0row", [P, P], f32, kind="ExternalInput").ap()
    ident_d = nc.dram_tensor("ident", [P, P], f32, kind="ExternalInput").ap()
    out = nc.dram_tensor("out", [N, F], f32, kind="ExternalOutput").ap()
    out_t = out.rearrange("(t p) f -> p t f", p=P)

    with ExitStack() as ctx:
        tc = ctx.enter_context(tile.TileContext(nc))
        consts = ctx.enter_context(tc.tile_pool(name="consts", bufs=1))
        afp = ctx.enter_context(tc.tile_pool(name="afp", bufs=8))
        abp = ctx.enter_context(tc.tile_pool(name="abp", bufs=5))
        big = ctx.enter_context(tc.tile_pool(name="big", bufs=1))
        small = ctx.enter_context(tc.tile_pool(name="small", bufs=4))
        ps_t = ctx.enter_context(tc.tile_pool(name="ps_t", bufs=2, space="PSUM"))
        ps_z = ctx.enter_context(tc.tile_pool(name="ps_z", bufs=4, space="PSUM"))

        # ---- input DMAs: x + consts on the ACT HWDGE ring so they run
        # alongside the adjacency strips on the SP ring (gpsimd/SWDGE is
        # avoided entirely -- its first use costs a ~15us drain) ---------
        ident_f = consts.tile([P, P], f32)
        nc.sync.dma_start(out=ident_f, in_=ident_d)
        v_f = consts.tile([P, K, F], f32)
        nc.sync.dma_start(out=v_f, in_=v_d.rearrange("k i o -> i k o"))
        e0_f = consts.tile([P, P], f32)
        nc.sync.dma_start(out=e0_f, in_=e0row_d)
        brow_f = consts.tile([P, F], f32)
        nc.sync.dma_start(out=brow_f, in_=brow_d)
        x_nat = big.tile([P, NT, F], f32)
        nc.scalar.dma_start(out=x_nat, in_=x)
        ident_bf = consts.tile([P, P], bf16)
        nc.scalar.copy(out=ident_bf, in_=ident_f)
        v_bf = consts.tile([P, K, F], bf16)
        nc.scalar.copy(out=v_bf, in_=v_f)
        e0_bf = consts.tile([P, P], bf16)
        nc.scalar.copy(out=e0_bf, in_=e0_f)
        brow_bf = consts.tile([P, F], bf16)
        nc.scalar.copy(out=brow_bf, in_=brow_f)
        eps_sb = consts.tile([P, 1], f32)
        nc.vector.memset(eps_sb, EPS)

        a_fs = {}

        def issue_strip_dma(r):
            a_f = afp.tile([P, N], f32, tag="af")
            nc.sync.dma_start(out=a_f, in_=adj[r * P:(r + 1) * P, :])
            a_fs[r] = a_f

        for r in range(8):
            issue_strip_dma(r)

        d_all = consts.tile([P, NT], f32)
        dsq_all = consts.tile([P, NT], f32)
        dinv = consts.tile([P, NT], f32)
        dinv2 = consts.tile([P, NT], f32)

        y0 = big.tile([P, NT, F], bf16)
        y1 = big.tile([P, NT, F], bf16)
        x_bf = big.tile([P, NT, F], bf16)
        xT = big.tile([P, NT, P], bf16)
        z1bf = big.tile([P, N], bf16)    # w1^T  [f, i]
        z2bf = big.tile([P, N], bf16)    # w2^T  [f, i]
        psa_sb = big.tile([P, NT, F], f32)  # x V0 + bsum, per i-chunk
        ats = big.tile([P, NT, N], bf16)    # [j, c, i] = A[i, j in block c]

        # ---- x prep: cast, then 16 PE transposes (warms the PE) --------
        nc.scalar.copy(out=x_bf, in_=x_nat)
        for g8 in range(2):
            pt = ps_t.tile([P, 8, P], f32, tag="t")
            for q in range(8):
                r = 8 * g8 + q
                nc.tensor.matmul(pt[:, q, :], lhsT=x_bf[:, r, :], rhs=ident_bf,
                                 start=True, stop=True)
            if g8 == 0:
                nc.vector.tensor_copy(out=xT[:, 0:8, :], in_=pt)
            else:
                nc.scalar.copy(out=xT[:, 8:16, :], in_=pt)

        # psa = xT V0 + bsum for all chunks (independent of A; runs in
        # whatever PE/ACT slack the stream leaves)
        def psa_pair(ch):
            pa = ps_t.tile([P, 2, F], f32, tag="t")
            for k in range(2):
                nc.tensor.matmul(pa[:, k, :], lhsT=xT[:, ch + k, :],
                                 rhs=v_bf[:, 0, :], start=True, stop=False,
                                 skip_group_check=True)
                nc.tensor.matmul(pa[:, k, :], lhsT=e0_bf, rhs=brow_bf,
                                 start=False, stop=True, skip_group_check=True)
            nc.scalar.copy(out=psa_sb[:, ch:ch + 2, :], in_=pa)

        # pass-1 bookkeeping: term (c, g) -> z1 column block g
        z1p = [ps_z.tile([P, 512], f32, tag=f"zg{g}", bufs=1,
                           name=f"z1p{g}") for g in range(NG)]
        p1_count = [0] * NG
        p1_issued = set()

        N_P1 = {g: NT for g in range(NG)}
        N_P1[3] = 2 * NT - 1  # 15 x 384-wide + 1 x 512 (c=15) + 15 x 128-wide

        def _p1_mm(c, g, col0, width):
            nc.tensor.matmul(z1p[g][:, col0:col0 + width],
                             lhsT=y0[:, c, :],
                             rhs=ats[:, c, g * 512 + col0:g * 512 + col0 + width],
                             start=(p1_count[g] == 0),
                             stop=(p1_count[g] == N_P1[g] - 1),
                             skip_group_check=True)
            p1_count[g] += 1

        def p1_term(c, g):
            if (c, g) in p1_issued:
                return
            p1_issued.add((c, g))
            _p1_mm(c, g, 0, 512)

        def p1_term_a(c):
            # group-3 columns from strips 12-14 only (available pre-tail)
            if (c, 3) in p1_issued:
                return
            p1_issued.add((c, 3))
            _p1_mm(c, 3, 0, 384)

        def p1_term_b(c):
            # group-3 remainder (strip 15 columns)
            if (c, "3b") in p1_issued:
                return
            p1_issued.add((c, "3b"))
            _p1_mm(c, 3, 384, 128)

        # ---- streaming phase ------------------------------------------
        # Engine FIFOs are strict; keep each engine\'s queue dependency-
        # monotone: DVE runs the casts (arrival order) plus tiny follow-ups,
        # ACT runs all PSUM evacs (PE completion order) plus dsq/y0/psa.
        for r in range(NT):
            a_f = a_fs[r]
            a_t = abp.tile([P, N], bf16, tag="a")
            d_r = small.tile([P, 1], f32, tag="d", bufs=8)
            nc.vector.tensor_scalar(
                out=a_t, in0=a_f, scalar1=1.0, scalar2=0.0, op0=OP.mult,
                op1=OP.add, accum_out=d_r)
            dsq_r = small.tile([P, 1], f32, tag="dsq", bufs=8)
            nc.scalar.activation(out=dsq_r, in_=d_r, func=AF.Sqrt, bias=eps_sb)
            nc.vector.reciprocal(out=dinv[:, r:r + 1], in_=dsq_r)
            nc.vector.tensor_mul(out=dinv2[:, r:r + 1], in0=dinv[:, r:r + 1],
                                 in1=dinv[:, r:r + 1])
            nc.scalar.activation(out=y0[:, r, :], in_=x_nat[:, r, :],
                                 func=AF.Identity, scale=dinv[:, r:r + 1])
            if r + 8 < NT:
                issue_strip_dma(r + 8)

            # transpose the strip: 16 (128x128) PE transposes vs identity;
            # two PSUM banks per evac op, all evacs on ACT
            for g8 in range(2):
                pt = ps_t.tile([P, 8, P], f32, tag="t")
                for q in range(8):
                    c = 8 * g8 + q
                    nc.tensor.matmul(pt[:, q, :], lhsT=a_t[:, c * P:(c + 1) * P],
                                     rhs=ident_bf, start=True, stop=True)
                nc.scalar.copy(
                    out=ats[:, 8 * g8:8 * g8 + 8, r * P:(r + 1) * P], in_=pt)

            if r < NT - 1:
                for gp in range((r + 1) // 4):
                    p1_term(r, gp)
                if r % 4 == 3:
                    g_new = r // 4
                    for c in range(r + 1):
                        p1_term(c, g_new)
                if r == NT - 2:
                    # strips 12-14 transposed: 384-wide group-3 terms
                    for c in range(NT - 1):
                        p1_term_a(c)
                if r >= 7:
                    psa_pair(2 * (r - 7))

        # ---- tail ------------------------------------------------------
        # Hand-interleaved so the PE never stalls on the DVE/ACT chains:
        # finish z1 group g with its (15, g) term, kick its evac, and keep
        # the PE busy on still-independent pass-1 rim terms while the
        # evac/transpose/scale chain for g runs; pass-2 terms follow as
        # their y1 blocks and z2 banks become ready.
        z2p = [None] * NG
        p2_count = [0] * NG
        p2_issued = set()

        def p2_term(c, gp):
            if (c, gp) in p2_issued or z2p[gp] is None:
                return
            p2_issued.add((c, gp))
            nc.tensor.matmul(z2p[gp], lhsT=y1[:, c, :],
                             rhs=ats[:, c, gp * 512:(gp + 1) * 512],
                             start=(p2_count[gp] == 0),
                             stop=(p2_count[gp] == NT - 1),
                             skip_group_check=True)
            p2_count[gp] += 1

        def z1_chain_pre(g):
            # evac on DVE/ACT (PE-independent)
            if g % 2 == 0:
                nc.vector.tensor_copy(out=z1bf[:, g * 512:(g + 1) * 512],
                                      in_=z1p[g])
            else:
                nc.scalar.copy(out=z1bf[:, g * 512:(g + 1) * 512], in_=z1p[g])

        def z1_chain_post(g):
            # 4 PE transposes -> natural, scale -> y1, free bank -> z2
            zn = ps_t.tile([P, 4, P], f32, tag="t", name=f"zn{g}")
            for q in range(4):
                c = 4 * g + q
                nc.tensor.matmul(zn[:, q, :], lhsT=z1bf[:, c * P:(c + 1) * P],
                                 rhs=ident_bf, start=True, stop=True)
            for q in range(4):
                c = 4 * g + q
                nc.vector.tensor_scalar(out=y1[:, c, :], in0=zn[:, q, :],
                                        scalar1=dinv2[:, c:c + 1], scalar2=None,
                                        op0=OP.mult)
            z2p[g] = ps_z.tile([P, 512], f32, tag=f"zg{g}", bufs=1,
                               name=f"z2p{g}")

        # all pass-1 terms not issued in-stream, ordered so z1 group 0
        # completes first; chains interleave to keep the PE fed
        pend = {gp: [c for c in range(NT) if (c, gp) not in p1_issued]
                for gp in range(3)}
        pend[3] = [c for c in range(NT) if (c, 3) not in p1_issued]

        for c in pend[0]:
            p1_term(c, 0)
        z1_chain_pre(0)
        for c in pend[1]:
            p1_term(c, 1)
        z1_chain_post(0)
        for c in pend[2]:
            p1_term(c, 2)
        z1_chain_pre(1)
        for c in range(4):
            p2_term(c, 0)
        rim_b = [c for c in range(NT - 1)]
        for c in pend[3]:
            p1_term(c, 3)
        for c in rim_b[:8]:
            p1_term_b(c)
        z1_chain_post(1)
        for c in rim_b[8:]:
            p1_term_b(c)
        z1_chain_pre(2)
        for gp in range(2):
            for c in range(8):
                p2_term(c, gp)
        z1_chain_post(2)
        z1_chain_pre(3)
        for gp in range(3):
            for c in range(12):
                p2_term(c, gp)
        z1_chain_post(3)
        # finish pass-2 one column group at a time; each group's output
        # (evac, psb matmuls, STT, relu, store) is issued while the PE
        # sweeps the next group's remaining terms, hiding the evac latency
        def z2_evac(g):
            if g % 2 == 0:
                nc.vector.tensor_copy(out=z2bf[:, g * 512:(g + 1) * 512],
                                      in_=z2p[g])
            else:
                nc.scalar.copy(out=z2bf[:, g * 512:(g + 1) * 512], in_=z2p[g])

        def out_group(g):
            og = small.tile([P, 4, F], f32, tag="og", bufs=2)
            tmp4 = small.tile([P, 4, F], f32, tag="tmp", bufs=2)
            for q in range(4):
                ch = 4 * g + q
                psb = ps_t.tile([P, F], f32, tag="t")
                nc.tensor.matmul(psb, lhsT=z1bf[:, ch * P:(ch + 1) * P],
                                 rhs=v_bf[:, 1, :], start=True, stop=False,
                                 skip_group_check=True)
                nc.tensor.matmul(psb, lhsT=z2bf[:, ch * P:(ch + 1) * P],
                                 rhs=v_bf[:, 2, :], start=False, stop=True,
                                 skip_group_check=True)
                nc.vector.scalar_tensor_tensor(
                    out=tmp4[:, q, :], in0=psb, scalar=dinv[:, ch:ch + 1],
                    in1=psa_sb[:, ch, :], op0=OP.mult, op1=OP.add)
            nc.scalar.activation(out=og, in_=tmp4, func=AF.Relu)
            nc.scalar.dma_start(out=out_t[:, 4 * g:4 * g + 4, :], in_=og)

        for c in range(NT):
            p2_term(c, 0)
        z2_evac(0)
        for c in range(NT):
            p2_term(c, 1)
        out_group(0)
        z2_evac(1)
        for c in range(NT):
            p2_term(c, 2)
        out_group(1)
        z2_evac(2)
        for c in range(NT):
            p2_term(c, 3)
        out_group(2)
        z2_evac(3)
        out_group(3)

    nc.compile()
    return nc

def _get_nc():
    if "nc" not in _cache:
        _cache["nc"] = _build_nc()
    return _cache["nc"]


def make_in_maps(x, adj, W, b):
    x = np.asarray(x, dtype=np.float32)
    # [B, N, F] -> [B, P, NT*F]: partition p holds x[t*128+p, :] for t in 0..15
    x_t = np.ascontiguousarray(
        x.reshape(B, NT, P, F).transpose(0, 2, 1, 3).reshape(B, P, NT * F))
    adj = np.ascontiguousarray(np.asarray(adj, dtype=np.float32))
    Wf = np.asarray(W, dtype=np.float32)
    bf = np.asarray(b, dtype=np.float32)
    V = np.ascontiguousarray(np.stack(
        [Wf[0] + Wf[1] + Wf[2], -(Wf[1] + 4.0 * Wf[2]), 2.0 * Wf[2]]))
    brow = np.zeros((P, F), dtype=np.float32)
    brow[0] = bf.sum(axis=0)
    e0row = np.zeros((P, P), dtype=np.float32)
    e0row[0] = 1.0
    ident = np.eye(P, dtype=np.float32)
    return [
        {"adj": adj[c], "x": x_t[c], "v": V, "brow": brow, "e0row": e0row,
         "ident": ident}
        for c in range(NCORES)
    ]


def run_raw(x, adj, W, b, **kwargs):
    from concourse import bass_utils

    nc = _get_nc()
    in_maps = make_in_maps(x, adj, W, b)
    res = bass_utils.run_bass_kernel_spmd(nc, in_maps,
                                          core_ids=list(range(NCORES)), **kwargs)
    out = np.stack([res.results[c]["out"] for c in range(NCORES)], axis=0)
    return out.astype(np.float32), res


def kernel(x, adj, W, b):
    out, _ = run_raw(x, adj, W, b)
    return out
